# revision 6
# baseline (speedup 1.0000x reference)
"""Trainium2 Bass kernel for nn_ActivationAndBiophysModel.

2-layer GRU (H=512) + FC + antagonist-muscle biophysics, T=512 steps, B=64.

Strategy notes (why this shape):
- The recurrence is strictly sequential over T=512; the per-step compute is
  small-matrix. On-chip collectives have a ~4.6us floor per call, so any
  cross-core exchange per timestep (1024 of them) costs more than the whole
  computation. Every core therefore runs the full model replicated; core 0's
  output is returned.
- Matmuls use "Form A": stationary operand = h^T (cheap to load, [K<=128, 64]),
  moving operand = weight panels streamed at 1 col/cycle. Cost is then just
  (weight elements / 128) cycles per step, independent of batch size <= 128.
- h^T is regenerated each step from the batch-major h via PE transpose.
- All biases are folded into ones-row matmuls (no extra vector ops).
- The biophysics integrator is algebraically collapsed into 9 fused DVE ops
  using scalar_tensor_tensor.
"""

import sys

for p in ("/opt/trn_rl_repo", "/opt/pypackages"):
    if p not in sys.path:
        sys.path.insert(0, p)

import numpy as np  # noqa: E402

B, T, IN, H, J = 64, 512, 16, 512, 8
HG = 3 * H  # 1536 gate rows

# muscle / joint constants (bilinearInit in the model)
K0, K1, L0m, L1m, Mm = 100.0, 2000.0, 0.06, 0.006, 0.05
Ij, Kj, Bj, DT = 0.004, 5.0, 0.3, 1.0 / 60.0

# collapsed integrator coefficients:
# s = a1+a0, d = a1-a0, p = s*d
# om' = ALPHA*om + BETA*d + GAMMA*p + DELTA*th + EPS*(s*th);  th' = th + DT*om'
_c = DT / Ij
ALPHA = 1.0 - _c * Bj
BETA = _c * Mm * (K0 * L1m + K1 * L0m)
GAMMA = _c * Mm * K1 * L1m
DELTA = _c * (-(2.0 * Mm * Mm * K0) - Kj)
EPS = _c * (-(Mm * Mm * K1))


def _build(nc, bass, tile, mybir, T_run, mv_dt, unroll_dma=8):
    """Emit the full unrolled program into nc."""
    f32 = mybir.dt.float32
    AF = mybir.ActivationFunctionType
    Alu = mybir.AluOpType

    def mm(out, lhsT, rhs, **kw):
        nc.tensor.matmul(out, lhsT.bitcast(mv_dt), rhs.bitcast(mv_dt), **kw)

    # ---- DRAM parameters -------------------------------------------------
    xT_d = nc.declare_dram_parameter("xT", [IN + 1, T_run, B], mv_dt, isOutput=False)
    w0x_d = nc.declare_dram_parameter("w0x", [IN + 1, HG], mv_dt, isOutput=False)
    w0h_d = nc.declare_dram_parameter("w0h", [4, 128, HG], mv_dt, isOutput=False)
    w1i_d = nc.declare_dram_parameter("w1i", [4, 128, HG], mv_dt, isOutput=False)
    w1h_d = nc.declare_dram_parameter("w1h", [4, 128, HG], mv_dt, isOutput=False)
    wfc_d = nc.declare_dram_parameter("wfc", [4, 128, 2 * J], mv_dt, isOutput=False)
    brows_d = nc.declare_dram_parameter("brows", [1, 2576 + B], mv_dt, isOutput=False)
    ident_d = nc.declare_dram_parameter("ident", [B, B], f32, isOutput=False)
    hb0_d = nc.declare_dram_parameter("hb0", [2, B, H], f32, isOutput=False)
    hT0_d = nc.declare_dram_parameter("hT0", [2, 4, 128, B], mv_dt, isOutput=False)
    th0_d = nc.declare_dram_parameter("th0", [B, J], f32, isOutput=False)
    om0_d = nc.declare_dram_parameter("om0", [B, J], f32, isOutput=False)
    out_d = nc.declare_dram_parameter("out", [B, T_run * J], f32, isOutput=True)

    # brows layout offsets
    OB1RZ, OB1IN, OB1HN, OB0HN, OBFC = 0, 1024, 1536, 2048, 2560

    with tile.TileContext(nc) as tc:
        with (
            tc.tile_pool(name="wpool", bufs=1) as wp,
            tc.tile_pool(name="xpool", bufs=unroll_dma) as xp,
            tc.tile_pool(name="state", bufs=2) as sp,
            tc.tile_pool(name="gates", bufs=2) as gp,
            tc.tile_pool(name="bp", bufs=2) as bp,
            tc.tile_pool(name="ppa", bufs=2, space="PSUM") as ppa,
            tc.tile_pool(name="ppb", bufs=2, space="PSUM") as ppb,
        ):
            # ---- load constants/weights once -----------------------------
            w0x = wp.tile([IN + 1, HG], mv_dt)
            nc.sync.dma_start(w0x[:], w0x_d[:])
            w0h = wp.tile([128, 4, HG], mv_dt)
            w1i = wp.tile([128, 4, HG], mv_dt)
            w1h = wp.tile([128, 4, HG], mv_dt)
            wfc = wp.tile([128, 4, 2 * J], mv_dt)
            for c in range(4):
                nc.sync.dma_start(w0h[:, c, :], w0h_d[c])
                nc.sync.dma_start(w1i[:, c, :], w1i_d[c])
                nc.sync.dma_start(w1h[:, c, :], w1h_d[c])
                nc.sync.dma_start(wfc[:, c, :], wfc_d[c])
            brows = wp.tile([1, 2576 + B], mv_dt)
            nc.sync.dma_start(brows[:], brows_d[:])
            ident = wp.tile([B, B], f32)
            nc.sync.dma_start(ident[:], ident_d[:])
            out_sb = wp.tile([B, T_run * J], f32)

            # ---- initial state -------------------------------------------
            h0b = sp.tile([B, H], f32, tag="h0b")
            h1b = sp.tile([B, H], f32, tag="h1b")
            nc.sync.dma_start(h0b[:], hb0_d[0])
            nc.sync.dma_start(h1b[:], hb0_d[1])
            h0T = sp.tile([128, 4 * B], mv_dt, tag="h0T")
            h1T = sp.tile([128, 4 * B], mv_dt, tag="h1T")
            for c in range(4):
                nc.sync.dma_start(h0T[:, c * B : (c + 1) * B], hT0_d[0, c])
                nc.sync.dma_start(h1T[:, c * B : (c + 1) * B], hT0_d[1, c])
            th_init = sp.tile([B, J], f32, tag="th")
            nc.sync.dma_start(th_init[:], th0_d[:])
            om = sp.tile([B, J], f32, tag="om")
            nc.sync.dma_start(om[:], om0_d[:])

            th_ap = th_init[:]

            # ---- time loop (fully unrolled) ------------------------------
            for t in range(T_run):
                xt = xp.tile([IN + 1, B], mv_dt, tag="xt")
                nc.sync.dma_start(xt[:], xT_d[:, t, :])

                # ===== layer 0 gates =====
                # rz: [B, 1024] = sigmoid inputs for r|z, fused gi+gh+biases
                ps_rz = ppa.tile([B, 1024], f32, tag="pa")
                for ns in range(2):
                    sl = ps_rz[:, ns * 512 : (ns + 1) * 512]
                    for c in range(4):  # h-part first: ready earliest
                        mm(
                            sl, h0T[:, c * B : (c + 1) * B],
                            w0h[:, c, ns * 512 : ns * 512 + 512],
                            start=(c == 0), stop=False,
                        )
                    mm(
                        sl, xt[:], w0x[:, ns * 512 : ns * 512 + 512],
                        start=False, stop=True,
                    )
                # hn | inn: [B, 1024]
                ps_ni = ppb.tile([B, 1024], f32, tag="pb")
                hn = ps_ni[:, 0:512]
                for c in range(4):
                    mm(
                        hn, h0T[:, c * B : (c + 1) * B],
                        w0h[:, c, 1024:1536], start=(c == 0), stop=False,
                    )
                mm(hn, brows[:, 2576 : 2576 + B], brows[:, OB0HN : OB0HN + 512],
                                 start=False, stop=True)
                inn = ps_ni[:, 512:1024]
                mm(inn, xt[:], w0x[:, 1024:1536],
                                 start=True, stop=True)

                # ===== layer 0 pointwise =====
                rz_s = gp.tile([B, 1024], f32, tag="rzs")
                nc.scalar.activation(rz_s[:], ps_rz[:], AF.Sigmoid)
                t0 = gp.tile([B, H], f32, tag="t0")
                nc.vector.tensor_mul(t0[:], rz_s[:, 0:512], hn)
                u0 = gp.tile([B, H], f32, tag="u0")
                nc.vector.tensor_add(u0[:], t0[:], inn)
                n0 = gp.tile([B, H], f32, tag="n0")
                nc.scalar.activation(n0[:], u0[:], AF.Tanh)
                d0 = gp.tile([B, H], f32, tag="d0")
                nc.vector.tensor_sub(d0[:], h0b[:], n0[:])
                zd0 = gp.tile([B, H], f32, tag="zd0")
                nc.vector.tensor_mul(zd0[:], rz_s[:, 512:1024], d0[:])
                h0b = sp.tile([B, H], f32, tag="h0b")
                nc.vector.tensor_add(h0b[:], n0[:], zd0[:])

                # transpose h0' -> h0T'
                ps_hT = ppa.tile([128, 4 * B], f32, tag="pa")
                for c in range(4):
                    nc.tensor.transpose(
                        ps_hT[:, c * B : (c + 1) * B],
                        h0b[:, c * 128 : (c + 1) * 128], ident[:],
                    )
                h0T = sp.tile([128, 4 * B], mv_dt, tag="h0T")
                nc.scalar.activation(h0T[:], ps_hT[:], AF.Copy)

                # ===== layer 1 gates =====
                ps_rz1 = ppa.tile([B, 1024], f32, tag="pa")
                for ns in range(2):
                    sl = ps_rz1[:, ns * 512 : (ns + 1) * 512]
                    for c in range(4):  # hh part first (h1T ready from t-1)
                        mm(
                            sl, h1T[:, c * B : (c + 1) * B],
                            w1h[:, c, ns * 512 : ns * 512 + 512],
                            start=(c == 0), stop=False,
                        )
                    for c in range(4):
                        mm(
                            sl, h0T[:, c * B : (c + 1) * B],
                            w1i[:, c, ns * 512 : ns * 512 + 512],
                            start=False, stop=False,
                        )
                    mm(
                        sl, brows[:, 2576 : 2576 + B], brows[:, OB1RZ + ns * 512 : OB1RZ + ns * 512 + 512],
                        start=False, stop=True,
                    )
                ps_ni1 = ppb.tile([B, 1024], f32, tag="pb")
                hn1 = ps_ni1[:, 0:512]
                for c in range(4):
                    mm(
                        hn1, h1T[:, c * B : (c + 1) * B],
                        w1h[:, c, 1024:1536], start=(c == 0), stop=False,
                    )
                mm(hn1, brows[:, 2576 : 2576 + B], brows[:, OB1HN : OB1HN + 512],
                                 start=False, stop=True)
                inn1 = ps_ni1[:, 512:1024]
                for c in range(4):
                    mm(
                        inn1, h0T[:, c * B : (c + 1) * B],
                        w1i[:, c, 1024:1536], start=(c == 0), stop=False,
                    )
                mm(inn1, brows[:, 2576 : 2576 + B], brows[:, OB1IN : OB1IN + 512],
                                 start=False, stop=True)

                # ===== layer 1 pointwise =====
                rz1_s = gp.tile([B, 1024], f32, tag="rz1s")
                nc.scalar.activation(rz1_s[:], ps_rz1[:], AF.Sigmoid)
                t1 = gp.tile([B, H], f32, tag="t1")
                nc.vector.tensor_mul(t1[:], rz1_s[:, 0:512], hn1)
                u1 = gp.tile([B, H], f32, tag="u1")
                nc.vector.tensor_add(u1[:], t1[:], inn1)
                n1 = gp.tile([B, H], f32, tag="n1")
                nc.scalar.activation(n1[:], u1[:], AF.Tanh)
                d1 = gp.tile([B, H], f32, tag="d1")
                nc.vector.tensor_sub(d1[:], h1b[:], n1[:])
                zd1 = gp.tile([B, H], f32, tag="zd1")
                nc.vector.tensor_mul(zd1[:], rz1_s[:, 512:1024], d1[:])
                h1b = sp.tile([B, H], f32, tag="h1b")
                nc.vector.tensor_add(h1b[:], n1[:], zd1[:])

                ps_hT1 = ppa.tile([128, 4 * B], f32, tag="pa")
                for c in range(4):
                    nc.tensor.transpose(
                        ps_hT1[:, c * B : (c + 1) * B],
                        h1b[:, c * 128 : (c + 1) * 128], ident[:],
                    )
                h1T = sp.tile([128, 4 * B], mv_dt, tag="h1T")
                nc.scalar.activation(h1T[:], ps_hT1[:], AF.Copy)

                # ===== FC + sigmoid =====
                ps_fc = ppb.tile([B, 1024], f32, tag="pb")
                fc = ps_fc[:, 0 : 2 * J]
                for c in range(4):
                    mm(fc, h1T[:, c * B : (c + 1) * B], wfc[:, c, :],
                                     start=(c == 0), stop=False)
                mm(fc, brows[:, 2576 : 2576 + B], brows[:, OBFC : OBFC + 2 * J],
                                 start=False, stop=True)
                a_s = bp.tile([B, J, 2], f32, tag="as")
                nc.scalar.activation(a_s[:], fc, AF.Sigmoid)

                # ===== biophysics (9 fused DVE ops) =====
                a0 = a_s[:, :, 0]
                a1 = a_s[:, :, 1]
                s_ = bp.tile([B, J], f32, tag="s")
                nc.vector.tensor_add(s_[:], a1, a0)
                dd = bp.tile([B, J], f32, tag="dd")
                nc.vector.tensor_sub(dd[:], a1, a0)
                p_ = bp.tile([B, J], f32, tag="p")
                nc.vector.tensor_mul(p_[:], s_[:], dd[:])
                w_ = bp.tile([B, J], f32, tag="w")
                nc.vector.scalar_tensor_tensor(
                    w_[:], dd[:], BETA / GAMMA, p_[:], Alu.mult, Alu.add)
                v_ = bp.tile([B, J], f32, tag="v")
                nc.vector.tensor_scalar(v_[:], s_[:], EPS, DELTA, Alu.mult, Alu.add)
                u2 = bp.tile([B, J], f32, tag="u2")
                nc.vector.tensor_mul(u2[:], v_[:], th_ap)
                q_ = bp.tile([B, J], f32, tag="q")
                nc.vector.scalar_tensor_tensor(
                    q_[:], w_[:], GAMMA, u2[:], Alu.mult, Alu.add)
                om_new = sp.tile([B, J], f32, tag="om")
                nc.vector.scalar_tensor_tensor(
                    om_new[:], om[:], ALPHA, q_[:], Alu.mult, Alu.add)
                om = om_new
                th_new = out_sb[:, t * J : (t + 1) * J]
                nc.vector.scalar_tensor_tensor(
                    th_new, om[:], DT, th_ap, Alu.mult, Alu.add)
                th_ap = th_new

            nc.sync.dma_start(out_d[:], out_sb[:])
    return nc


_NC_CACHE = {}


def _get_nc(T_run):
    key = T_run
    if key in _NC_CACHE:
        return _NC_CACHE[key]
    from concourse import bass, bacc, tile

    mybir = bass.mybir
    mv_dt = mybir.dt.float32r  # fp32 data, 1 cycle/row streaming for N>=256
    nc = bacc.Bacc(None, target_bir_lowering=False)
    _build(nc, bass, tile, mybir, T_run, mv_dt)
    nc.compile()
    _NC_CACHE[key] = nc
    return nc


def _prep_inputs(x, W_ih0, W_hh0, b_ih0, b_hh0, W_ih1, W_hh1, b_ih1, b_hh1,
                 fc_W, fc_b, h0, theta0, omega0):
    T_run = x.shape[1]
    f = np.float32
    xT = np.concatenate(
        [np.ascontiguousarray(x.transpose(2, 1, 0)),
         np.ones((1, T_run, B), f)], axis=0).astype(f)  # [17, T, 64]
    b0rz = (b_ih0 + b_hh0)[:1024]
    w0x = np.concatenate(
        [W_ih0.T, np.concatenate([b0rz, b_ih0[1024:]])[None, :]], axis=0
    ).astype(f)  # [17, 1536]
    w0h = np.ascontiguousarray(W_hh0.T.reshape(4, 128, HG)).astype(f)
    w1i = np.ascontiguousarray(W_ih1.T.reshape(4, 128, HG)).astype(f)
    w1h = np.ascontiguousarray(W_hh1.T.reshape(4, 128, HG)).astype(f)
    wfc = np.ascontiguousarray(fc_W.T.reshape(4, 128, 2 * J)).astype(f)
    brows = np.zeros((1, 2576 + B), f)
    brows[0, 2576:] = 1.0
    brows[0, 0:1024] = (b_ih1 + b_hh1)[:1024]
    brows[0, 1024:1536] = b_ih1[1024:]
    brows[0, 1536:2048] = b_hh1[1024:]
    brows[0, 2048:2560] = b_hh0[1024:]
    brows[0, 2560:2576] = fc_b
    hT0 = np.ascontiguousarray(
        np.stack([h0[0].T.reshape(4, 128, B), h0[1].T.reshape(4, 128, B)])
    ).astype(f)
    return {
        "xT": xT, "w0x": w0x, "w0h": w0h, "w1i": w1i, "w1h": w1h,
        "wfc": wfc, "brows": brows, "ident": np.eye(B, dtype=f),
        "hb0": h0.astype(f), "hT0": hT0,
        "th0": theta0.astype(f), "om0": omega0.astype(f),
    }


def _install_loud_hook():
    """Surface compile-hook exceptions (XLA otherwise swallows them)."""
    import traceback

    from concourse import bass2jax

    if getattr(bass2jax, "_loud_hook_installed", False):
        return
    orig = bass2jax.neuronx_cc_hook

    def loud(*a, **k):
        try:
            return orig(*a, **k)
        except BaseException:
            traceback.print_exc()
            raise

    bass2jax.neuronx_cc_hook = loud
    bass2jax._loud_hook_installed = True


def run(inputs, **spmd_kwargs):
    from concourse.bass_utils import run_bass_kernel_spmd

    _install_loud_hook()

    inputs = {k: np.asarray(v) for k, v in inputs.items()}
    T_run = inputs["x"].shape[1]
    nc = _get_nc(T_run)
    in_map = _prep_inputs(**inputs)
    res = run_bass_kernel_spmd(nc, [in_map] * 8, core_ids=list(range(8)),
                               **spmd_kwargs)
    out = res.results[0]["out"].reshape(B, T_run, J).astype(np.float32)
    return out, res


def kernel(**inputs):
    return run(inputs)[0]


if __name__ == "__main__":
    rs = np.random.RandomState(0)
    demo = {
        "x": rs.randn(B, 8, IN).astype(np.float32),
        "W_ih0": 0.04 * rs.randn(HG, IN).astype(np.float32),
        "W_hh0": 0.04 * rs.randn(HG, H).astype(np.float32),
        "b_ih0": 0.04 * rs.randn(HG).astype(np.float32),
        "b_hh0": 0.04 * rs.randn(HG).astype(np.float32),
        "W_ih1": 0.04 * rs.randn(HG, H).astype(np.float32),
        "W_hh1": 0.04 * rs.randn(HG, H).astype(np.float32),
        "b_ih1": 0.04 * rs.randn(HG).astype(np.float32),
        "b_hh1": 0.04 * rs.randn(HG).astype(np.float32),
        "fc_W": 0.04 * rs.randn(2 * J, H).astype(np.float32),
        "fc_b": 0.04 * rs.randn(2 * J).astype(np.float32),
        "h0": np.zeros((2, B, H), np.float32),
        "theta0": np.zeros((B, J), np.float32),
        "omega0": np.zeros((B, J), np.float32),
    }
    print(kernel(**demo).shape)


# revision 8
# speedup vs baseline: 1.0433x; 1.0433x over previous
"""Trainium2 Bass kernel for nn_ActivationAndBiophysModel.

2-layer GRU (H=512) + FC + antagonist-muscle biophysics, T=512 steps, B=64.

Strategy notes (why this shape):
- The recurrence is strictly sequential over T=512; the per-step compute is
  small-matrix. On-chip collectives have a ~4.6us floor per call, so any
  cross-core exchange per timestep (1024 of them) costs more than the whole
  computation. Every core therefore runs the full model replicated; core 0's
  output is returned.
- Matmuls use "Form A": stationary operand = h^T (cheap to load, [K<=128, 64]),
  moving operand = weight panels streamed at 1 col/cycle. Cost is then just
  (weight elements / 128) cycles per step, independent of batch size <= 128.
- h^T is regenerated each step from the batch-major h via PE transpose.
- All biases are folded into ones-row matmuls (no extra vector ops).
- The biophysics integrator is algebraically collapsed into 9 fused DVE ops
  using scalar_tensor_tensor.
"""

import sys

for p in ("/opt/trn_rl_repo", "/opt/pypackages"):
    if p not in sys.path:
        sys.path.insert(0, p)

import numpy as np  # noqa: E402

B, T, IN, H, J = 64, 512, 16, 512, 8
HG = 3 * H  # 1536 gate rows

# muscle / joint constants (bilinearInit in the model)
K0, K1, L0m, L1m, Mm = 100.0, 2000.0, 0.06, 0.006, 0.05
Ij, Kj, Bj, DT = 0.004, 5.0, 0.3, 1.0 / 60.0

# collapsed integrator coefficients:
# s = a1+a0, d = a1-a0, p = s*d
# om' = ALPHA*om + BETA*d + GAMMA*p + DELTA*th + EPS*(s*th);  th' = th + DT*om'
_c = DT / Ij
ALPHA = 1.0 - _c * Bj
BETA = _c * Mm * (K0 * L1m + K1 * L0m)
GAMMA = _c * Mm * K1 * L1m
DELTA = _c * (-(2.0 * Mm * Mm * K0) - Kj)
EPS = _c * (-(Mm * Mm * K1))


def _build(nc, bass, tile, mybir, T_run, mv_dt, unroll_dma=8):
    """Emit the full unrolled program into nc."""
    f32 = mybir.dt.float32
    AF = mybir.ActivationFunctionType
    Alu = mybir.AluOpType

    def mm(out, lhsT, rhs, **kw):
        nc.tensor.matmul(out, lhsT.bitcast(mv_dt), rhs.bitcast(mv_dt), **kw)

    # ---- DRAM parameters -------------------------------------------------
    xT_d = nc.declare_dram_parameter("xT", [IN + 1, T_run, B], mv_dt, isOutput=False)
    w0x_d = nc.declare_dram_parameter("w0x", [IN + 1, HG], mv_dt, isOutput=False)
    w0h_d = nc.declare_dram_parameter("w0h", [4, 128, HG], mv_dt, isOutput=False)
    w1i_d = nc.declare_dram_parameter("w1i", [4, 128, HG], mv_dt, isOutput=False)
    w1h_d = nc.declare_dram_parameter("w1h", [4, 128, HG], mv_dt, isOutput=False)
    wfc_d = nc.declare_dram_parameter("wfc", [4, 128, 2 * J], mv_dt, isOutput=False)
    brows_d = nc.declare_dram_parameter("brows", [1, 2576 + B], mv_dt, isOutput=False)
    ident_d = nc.declare_dram_parameter("ident", [B, B], f32, isOutput=False)
    hb0_d = nc.declare_dram_parameter("hb0", [2, B, H], f32, isOutput=False)
    hT0_d = nc.declare_dram_parameter("hT0", [2, 4, 128, B], mv_dt, isOutput=False)
    th0_d = nc.declare_dram_parameter("th0", [B, J], f32, isOutput=False)
    om0_d = nc.declare_dram_parameter("om0", [B, J], f32, isOutput=False)
    out_d = nc.declare_dram_parameter("out", [B, T_run * J], f32, isOutput=True)

    # brows layout offsets
    OB1RZ, OB1IN, OB1HN, OB0HN, OBFC = 0, 1024, 1536, 2048, 2560

    with tile.TileContext(nc) as tc:
        with (
            tc.tile_pool(name="wpool", bufs=1) as wp,
            tc.tile_pool(name="xpool", bufs=unroll_dma) as xp,
            tc.tile_pool(name="state", bufs=2) as sp,
            tc.tile_pool(name="gates", bufs=2) as gp,
            tc.tile_pool(name="bp", bufs=2) as bp,
            tc.tile_pool(name="ppa", bufs=2, space="PSUM") as ppa,
            tc.tile_pool(name="ppb", bufs=2, space="PSUM") as ppb,
        ):
            # ---- load constants/weights once -----------------------------
            w0x = wp.tile([IN + 1, HG], mv_dt)
            nc.sync.dma_start(w0x[:], w0x_d[:])
            w0h = wp.tile([128, 4, HG], mv_dt)
            w1i = wp.tile([128, 4, HG], mv_dt)
            w1h = wp.tile([128, 4, HG], mv_dt)
            wfc = wp.tile([128, 4, 2 * J], mv_dt)
            for c in range(4):
                nc.sync.dma_start(w0h[:, c, :], w0h_d[c])
                nc.sync.dma_start(w1i[:, c, :], w1i_d[c])
                nc.sync.dma_start(w1h[:, c, :], w1h_d[c])
                nc.sync.dma_start(wfc[:, c, :], wfc_d[c])
            brows = wp.tile([1, 2576 + B], mv_dt)
            nc.sync.dma_start(brows[:], brows_d[:])
            ident = wp.tile([B, B], f32)
            nc.sync.dma_start(ident[:], ident_d[:])
            out_sb = wp.tile([B, T_run * J], f32)

            # ---- initial state -------------------------------------------
            h0b = sp.tile([B, H], f32, tag="h0b")
            h1b = sp.tile([B, H], f32, tag="h1b")
            nc.sync.dma_start(h0b[:], hb0_d[0])
            nc.sync.dma_start(h1b[:], hb0_d[1])
            h0T = sp.tile([128, 4 * B], mv_dt, tag="h0T")
            h1T = sp.tile([128, 4 * B], mv_dt, tag="h1T")
            for c in range(4):
                nc.sync.dma_start(h0T[:, c * B : (c + 1) * B], hT0_d[0, c])
                nc.sync.dma_start(h1T[:, c * B : (c + 1) * B], hT0_d[1, c])
            th_init = sp.tile([B, J], f32, tag="th")
            nc.sync.dma_start(th_init[:], th0_d[:])
            om = sp.tile([B, J], f32, tag="om")
            nc.sync.dma_start(om[:], om0_d[:])

            th_ap = th_init[:]

            # ---- time loop (fully unrolled) ------------------------------
            for t in range(T_run):
                xt = xp.tile([IN + 1, B], mv_dt, tag="xt")
                nc.sync.dma_start(xt[:], xT_d[:, t, :])

                # ===== layer 0 gates =====
                # chunk-major order: consecutive matmuls share the stationary
                # operand so walrus ldw-opt can elide redundant LDWEIGHTS.
                ps_rz = ppa.tile([B, 1024], f32, tag="pa")
                ps_ni = ppb.tile([B, 1024], f32, tag="pb")
                hn = ps_ni[:, 0:512]
                inn = ps_ni[:, 512:1024]
                for c in range(4):
                    st = h0T[:, c * B : (c + 1) * B]
                    for ns in range(2):
                        mm(ps_rz[:, ns * 512 : (ns + 1) * 512], st,
                           w0h[:, c, ns * 512 : ns * 512 + 512],
                           start=(c == 0), stop=False)
                    mm(hn, st, w0h[:, c, 1024:1536], start=(c == 0), stop=False)
                for ns in range(2):
                    mm(ps_rz[:, ns * 512 : (ns + 1) * 512], xt[:],
                       w0x[:, ns * 512 : ns * 512 + 512], start=False, stop=True)
                mm(inn, xt[:], w0x[:, 1024:1536], start=True, stop=True)
                mm(hn, brows[:, 2576 : 2576 + B], brows[:, OB0HN : OB0HN + 512],
                   start=False, stop=True)

                # ===== layer 0 pointwise =====
                rz_s = gp.tile([B, 1024], f32, tag="rzs")
                nc.scalar.activation(rz_s[:], ps_rz[:], AF.Sigmoid)
                t0 = gp.tile([B, H], f32, tag="t0")
                nc.vector.tensor_mul(t0[:], rz_s[:, 0:512], hn)
                u0 = gp.tile([B, H], f32, tag="u0")
                nc.vector.tensor_add(u0[:], t0[:], inn)
                n0 = gp.tile([B, H], f32, tag="n0")
                nc.scalar.activation(n0[:], u0[:], AF.Tanh)
                d0 = gp.tile([B, H], f32, tag="d0")
                nc.vector.tensor_sub(d0[:], h0b[:], n0[:])
                zd0 = gp.tile([B, H], f32, tag="zd0")
                nc.vector.tensor_mul(zd0[:], rz_s[:, 512:1024], d0[:])
                h0b = sp.tile([B, H], f32, tag="h0b")
                nc.vector.tensor_add(h0b[:], n0[:], zd0[:])

                # transpose h0' -> h0T'
                ps_hT = ppa.tile([128, 4 * B], f32, tag="pa")
                for c in range(4):
                    nc.tensor.transpose(
                        ps_hT[:, c * B : (c + 1) * B],
                        h0b[:, c * 128 : (c + 1) * 128], ident[:],
                    )
                h0T = sp.tile([128, 4 * B], mv_dt, tag="h0T")
                nc.scalar.activation(h0T[:], ps_hT[:], AF.Copy)

                # ===== layer 1 gates =====
                ps_rz1 = ppa.tile([B, 1024], f32, tag="pa")
                ps_ni1 = ppb.tile([B, 1024], f32, tag="pb")
                hn1 = ps_ni1[:, 0:512]
                inn1 = ps_ni1[:, 512:1024]
                for c in range(4):  # hh part first (h1T ready from t-1)
                    st = h1T[:, c * B : (c + 1) * B]
                    for ns in range(2):
                        mm(ps_rz1[:, ns * 512 : (ns + 1) * 512], st,
                           w1h[:, c, ns * 512 : ns * 512 + 512],
                           start=(c == 0), stop=False)
                    mm(hn1, st, w1h[:, c, 1024:1536], start=(c == 0), stop=False)
                for c in range(4):
                    st = h0T[:, c * B : (c + 1) * B]
                    for ns in range(2):
                        mm(ps_rz1[:, ns * 512 : (ns + 1) * 512], st,
                           w1i[:, c, ns * 512 : ns * 512 + 512],
                           start=False, stop=False)
                    mm(inn1, st, w1i[:, c, 1024:1536], start=(c == 0), stop=False)
                onesap = brows[:, 2576 : 2576 + B]
                for ns in range(2):
                    mm(ps_rz1[:, ns * 512 : (ns + 1) * 512], onesap,
                       brows[:, OB1RZ + ns * 512 : OB1RZ + ns * 512 + 512],
                       start=False, stop=True)
                mm(hn1, onesap, brows[:, OB1HN : OB1HN + 512], start=False, stop=True)
                mm(inn1, onesap, brows[:, OB1IN : OB1IN + 512], start=False, stop=True)

                # ===== layer 1 pointwise =====
                rz1_s = gp.tile([B, 1024], f32, tag="rz1s")
                nc.scalar.activation(rz1_s[:], ps_rz1[:], AF.Sigmoid)
                t1 = gp.tile([B, H], f32, tag="t1")
                nc.vector.tensor_mul(t1[:], rz1_s[:, 0:512], hn1)
                u1 = gp.tile([B, H], f32, tag="u1")
                nc.vector.tensor_add(u1[:], t1[:], inn1)
                n1 = gp.tile([B, H], f32, tag="n1")
                nc.scalar.activation(n1[:], u1[:], AF.Tanh)
                d1 = gp.tile([B, H], f32, tag="d1")
                nc.vector.tensor_sub(d1[:], h1b[:], n1[:])
                zd1 = gp.tile([B, H], f32, tag="zd1")
                nc.vector.tensor_mul(zd1[:], rz1_s[:, 512:1024], d1[:])
                h1b = sp.tile([B, H], f32, tag="h1b")
                nc.vector.tensor_add(h1b[:], n1[:], zd1[:])

                ps_hT1 = ppa.tile([128, 4 * B], f32, tag="pa")
                for c in range(4):
                    nc.tensor.transpose(
                        ps_hT1[:, c * B : (c + 1) * B],
                        h1b[:, c * 128 : (c + 1) * 128], ident[:],
                    )
                h1T = sp.tile([128, 4 * B], mv_dt, tag="h1T")
                nc.scalar.activation(h1T[:], ps_hT1[:], AF.Copy)

                # ===== FC + sigmoid =====
                ps_fc = ppb.tile([B, 1024], f32, tag="pb")
                fc = ps_fc[:, 0 : 2 * J]
                for c in range(4):
                    mm(fc, h1T[:, c * B : (c + 1) * B], wfc[:, c, :],
                                     start=(c == 0), stop=False)
                mm(fc, brows[:, 2576 : 2576 + B], brows[:, OBFC : OBFC + 2 * J],
                                 start=False, stop=True)
                a_s = bp.tile([B, J, 2], f32, tag="as")
                nc.scalar.activation(a_s[:], fc, AF.Sigmoid)

                # ===== biophysics (9 fused DVE ops) =====
                a0 = a_s[:, :, 0]
                a1 = a_s[:, :, 1]
                s_ = bp.tile([B, J], f32, tag="s")
                nc.vector.tensor_add(s_[:], a1, a0)
                dd = bp.tile([B, J], f32, tag="dd")
                nc.vector.tensor_sub(dd[:], a1, a0)
                p_ = bp.tile([B, J], f32, tag="p")
                nc.vector.tensor_mul(p_[:], s_[:], dd[:])
                w_ = bp.tile([B, J], f32, tag="w")
                nc.vector.scalar_tensor_tensor(
                    w_[:], dd[:], BETA / GAMMA, p_[:], Alu.mult, Alu.add)
                v_ = bp.tile([B, J], f32, tag="v")
                nc.vector.tensor_scalar(v_[:], s_[:], EPS, DELTA, Alu.mult, Alu.add)
                u2 = bp.tile([B, J], f32, tag="u2")
                nc.vector.tensor_mul(u2[:], v_[:], th_ap)
                q_ = bp.tile([B, J], f32, tag="q")
                nc.vector.scalar_tensor_tensor(
                    q_[:], w_[:], GAMMA, u2[:], Alu.mult, Alu.add)
                om_new = sp.tile([B, J], f32, tag="om")
                nc.vector.scalar_tensor_tensor(
                    om_new[:], om[:], ALPHA, q_[:], Alu.mult, Alu.add)
                om = om_new
                th_new = out_sb[:, t * J : (t + 1) * J]
                nc.vector.scalar_tensor_tensor(
                    th_new, om[:], DT, th_ap, Alu.mult, Alu.add)
                th_ap = th_new

            nc.sync.dma_start(out_d[:], out_sb[:])
    return nc


_NC_CACHE = {}


def _get_nc(T_run):
    key = T_run
    if key in _NC_CACHE:
        return _NC_CACHE[key]
    from concourse import bass, bacc, tile

    mybir = bass.mybir
    mv_dt = mybir.dt.float32r  # fp32 data, 1 cycle/row streaming for N>=256
    nc = bacc.Bacc(None, target_bir_lowering=False)
    _build(nc, bass, tile, mybir, T_run, mv_dt)
    nc.compile()
    _NC_CACHE[key] = nc
    return nc


def _prep_inputs(x, W_ih0, W_hh0, b_ih0, b_hh0, W_ih1, W_hh1, b_ih1, b_hh1,
                 fc_W, fc_b, h0, theta0, omega0):
    T_run = x.shape[1]
    f = np.float32
    xT = np.concatenate(
        [np.ascontiguousarray(x.transpose(2, 1, 0)),
         np.ones((1, T_run, B), f)], axis=0).astype(f)  # [17, T, 64]
    b0rz = (b_ih0 + b_hh0)[:1024]
    w0x = np.concatenate(
        [W_ih0.T, np.concatenate([b0rz, b_ih0[1024:]])[None, :]], axis=0
    ).astype(f)  # [17, 1536]
    w0h = np.ascontiguousarray(W_hh0.T.reshape(4, 128, HG)).astype(f)
    w1i = np.ascontiguousarray(W_ih1.T.reshape(4, 128, HG)).astype(f)
    w1h = np.ascontiguousarray(W_hh1.T.reshape(4, 128, HG)).astype(f)
    wfc = np.ascontiguousarray(fc_W.T.reshape(4, 128, 2 * J)).astype(f)
    brows = np.zeros((1, 2576 + B), f)
    brows[0, 2576:] = 1.0
    brows[0, 0:1024] = (b_ih1 + b_hh1)[:1024]
    brows[0, 1024:1536] = b_ih1[1024:]
    brows[0, 1536:2048] = b_hh1[1024:]
    brows[0, 2048:2560] = b_hh0[1024:]
    brows[0, 2560:2576] = fc_b
    hT0 = np.ascontiguousarray(
        np.stack([h0[0].T.reshape(4, 128, B), h0[1].T.reshape(4, 128, B)])
    ).astype(f)
    return {
        "xT": xT, "w0x": w0x, "w0h": w0h, "w1i": w1i, "w1h": w1h,
        "wfc": wfc, "brows": brows, "ident": np.eye(B, dtype=f),
        "hb0": h0.astype(f), "hT0": hT0,
        "th0": theta0.astype(f), "om0": omega0.astype(f),
    }


def _install_loud_hook():
    """Surface compile-hook exceptions (XLA otherwise swallows them)."""
    import traceback

    from concourse import bass2jax

    if getattr(bass2jax, "_loud_hook_installed", False):
        return
    orig = bass2jax.neuronx_cc_hook

    def loud(*a, **k):
        try:
            return orig(*a, **k)
        except BaseException:
            traceback.print_exc()
            raise

    bass2jax.neuronx_cc_hook = loud
    bass2jax._loud_hook_installed = True

    # Enable walrus's LDWEIGHTS-dedup pass (concourse hardcodes it off).
    # Our matmuls are chunk-major so consecutive MMs share the stationary
    # operand; the dedup removes ~half the weight-load traffic.
    import os

    if os.environ.get("KERNEL_LDW_OPT", "1") == "1":
        from concourse import bass_utils as _bu

        if not getattr(_bu, "_ldw_patch", False):
            _orig_rc = _bu.run_command

            def _rc(cmd, **kw):
                cmd = [c.replace("--enable-ldw-opt=false", "--enable-ldw-opt=true")
                       if isinstance(c, str) else c for c in cmd]
                return _orig_rc(cmd, **kw)

            _bu.run_command = _rc
            _bu._ldw_patch = True


def run(inputs, **spmd_kwargs):
    from concourse.bass_utils import run_bass_kernel_spmd

    _install_loud_hook()

    inputs = {k: np.asarray(v) for k, v in inputs.items()}
    T_run = inputs["x"].shape[1]
    nc = _get_nc(T_run)
    in_map = _prep_inputs(**inputs)
    res = run_bass_kernel_spmd(nc, [in_map] * 8, core_ids=list(range(8)),
                               **spmd_kwargs)
    out = res.results[0]["out"].reshape(B, T_run, J).astype(np.float32)
    return out, res


def kernel(**inputs):
    return run(inputs)[0]


if __name__ == "__main__":
    rs = np.random.RandomState(0)
    demo = {
        "x": rs.randn(B, 8, IN).astype(np.float32),
        "W_ih0": 0.04 * rs.randn(HG, IN).astype(np.float32),
        "W_hh0": 0.04 * rs.randn(HG, H).astype(np.float32),
        "b_ih0": 0.04 * rs.randn(HG).astype(np.float32),
        "b_hh0": 0.04 * rs.randn(HG).astype(np.float32),
        "W_ih1": 0.04 * rs.randn(HG, H).astype(np.float32),
        "W_hh1": 0.04 * rs.randn(HG, H).astype(np.float32),
        "b_ih1": 0.04 * rs.randn(HG).astype(np.float32),
        "b_hh1": 0.04 * rs.randn(HG).astype(np.float32),
        "fc_W": 0.04 * rs.randn(2 * J, H).astype(np.float32),
        "fc_b": 0.04 * rs.randn(2 * J).astype(np.float32),
        "h0": np.zeros((2, B, H), np.float32),
        "theta0": np.zeros((B, J), np.float32),
        "omega0": np.zeros((B, J), np.float32),
    }
    print(kernel(**demo).shape)


# revision 11
# speedup vs baseline: 1.4087x; 1.3503x over previous
"""Trainium2 Bass kernel for nn_ActivationAndBiophysModel.

2-layer GRU (H=512) + FC + antagonist-muscle biophysics, T=512 steps, B=64.

Strategy notes (why this shape):
- The recurrence is strictly sequential over T=512; the per-step compute is
  small-matrix. On-chip collectives have a ~4.6us floor per call, so any
  cross-core exchange per timestep (1024 of them) costs more than the whole
  computation. Every core therefore runs the full model replicated; core 0's
  output is returned.
- Matmuls use "Form A": stationary operand = h^T (cheap to load, [K<=128, 64]),
  moving operand = weight panels streamed at 1 col/cycle. Cost is then just
  (weight elements / 128) cycles per step, independent of batch size <= 128.
- h^T is regenerated each step from the batch-major h via PE transpose.
- All biases are folded into ones-row matmuls (no extra vector ops).
- The biophysics integrator is algebraically collapsed into 9 fused DVE ops
  using scalar_tensor_tensor.
"""

import sys

for p in ("/opt/trn_rl_repo", "/opt/pypackages"):
    if p not in sys.path:
        sys.path.insert(0, p)

import numpy as np  # noqa: E402

B, T, IN, H, J = 64, 512, 16, 512, 8
HG = 3 * H  # 1536 gate rows

# muscle / joint constants (bilinearInit in the model)
K0, K1, L0m, L1m, Mm = 100.0, 2000.0, 0.06, 0.006, 0.05
Ij, Kj, Bj, DT = 0.004, 5.0, 0.3, 1.0 / 60.0

# collapsed integrator coefficients:
# s = a1+a0, d = a1-a0, p = s*d
# om' = ALPHA*om + BETA*d + GAMMA*p + DELTA*th + EPS*(s*th);  th' = th + DT*om'
_c = DT / Ij
ALPHA = 1.0 - _c * Bj
BETA = _c * Mm * (K0 * L1m + K1 * L0m)
GAMMA = _c * Mm * K1 * L1m
DELTA = _c * (-(2.0 * Mm * Mm * K0) - Kj)
EPS = _c * (-(Mm * Mm * K1))


def _build(nc, bass, tile, mybir, T_run, mv_dt, unroll_dma=8):
    """Emit the full unrolled program into nc."""
    f32 = mybir.dt.float32
    AF = mybir.ActivationFunctionType
    Alu = mybir.AluOpType

    def mm(out, lhsT, rhs, **kw):
        nc.tensor.matmul(out, lhsT.bitcast(mv_dt), rhs.bitcast(mv_dt), **kw)

    # ---- DRAM parameters -------------------------------------------------
    xT_d = nc.declare_dram_parameter("xT", [IN + 1, T_run, B], mv_dt, isOutput=False)
    w0x_d = nc.declare_dram_parameter("w0x", [IN + 1, HG], mv_dt, isOutput=False)
    w0h_d = nc.declare_dram_parameter("w0h", [4, 128, HG], mv_dt, isOutput=False)
    w1i_d = nc.declare_dram_parameter("w1i", [4, 128, HG], mv_dt, isOutput=False)
    w1h_d = nc.declare_dram_parameter("w1h", [4, 128, HG], mv_dt, isOutput=False)
    wfc_d = nc.declare_dram_parameter("wfc", [4, 128, 2 * J], mv_dt, isOutput=False)
    brows_d = nc.declare_dram_parameter("brows", [1, 2576 + B], mv_dt, isOutput=False)
    ident_d = nc.declare_dram_parameter("ident", [B, B], f32, isOutput=False)
    hb0_d = nc.declare_dram_parameter("hb0", [2, B, H], f32, isOutput=False)
    hT0_d = nc.declare_dram_parameter("hT0", [2, 4, 128, B], mv_dt, isOutput=False)
    th0_d = nc.declare_dram_parameter("th0", [B, J], f32, isOutput=False)
    om0_d = nc.declare_dram_parameter("om0", [B, J], f32, isOutput=False)
    out_d = nc.declare_dram_parameter("out", [B, T_run * J], f32, isOutput=True)

    # brows layout offsets
    OB1RZ, OB1IN, OB1HN, OB0HN, OBFC = 0, 1024, 1536, 2048, 2560

    with tile.TileContext(nc) as tc:
        with (
            tc.tile_pool(name="wpool", bufs=1) as wp,
            tc.tile_pool(name="xpool", bufs=unroll_dma) as xp,
            tc.tile_pool(name="state", bufs=2) as sp,
            tc.tile_pool(name="gates", bufs=2) as gp,
            tc.tile_pool(name="bp", bufs=2) as bp,
            tc.tile_pool(name="prz", bufs=2, space="PSUM") as prz,
            tc.tile_pool(name="pni", bufs=1, space="PSUM") as pni,
            tc.tile_pool(name="pht", bufs=2, space="PSUM") as pht,
        ):
            # ---- load constants/weights once -----------------------------
            w0x = wp.tile([IN + 1, HG], mv_dt)
            nc.sync.dma_start(w0x[:], w0x_d[:])
            w0h = wp.tile([128, 4, HG], mv_dt)
            w1i = wp.tile([128, 4, HG], mv_dt)
            w1h = wp.tile([128, 4, HG], mv_dt)
            wfc = wp.tile([128, 4, 2 * J], mv_dt)
            for c in range(4):
                nc.sync.dma_start(w0h[:, c, :], w0h_d[c])
                nc.sync.dma_start(w1i[:, c, :], w1i_d[c])
                nc.sync.dma_start(w1h[:, c, :], w1h_d[c])
                nc.sync.dma_start(wfc[:, c, :], wfc_d[c])
            brows = wp.tile([1, 2576 + B], mv_dt)
            nc.sync.dma_start(brows[:], brows_d[:])
            ident = wp.tile([B, B], f32)
            nc.sync.dma_start(ident[:], ident_d[:])
            out_sb = wp.tile([B, T_run * J], f32)

            # ---- initial state -------------------------------------------
            h0b = sp.tile([B, H], f32, tag="h0b")
            h1b = sp.tile([B, H], f32, tag="h1b")
            nc.sync.dma_start(h0b[:], hb0_d[0])
            nc.sync.dma_start(h1b[:], hb0_d[1])
            h0T = sp.tile([128, 4 * B], mv_dt, tag="h0T")
            h1T = sp.tile([128, 4 * B], mv_dt, tag="h1T")
            for c in range(4):
                nc.sync.dma_start(h0T[:, c * B : (c + 1) * B], hT0_d[0, c])
                nc.sync.dma_start(h1T[:, c * B : (c + 1) * B], hT0_d[1, c])
            th_init = sp.tile([B, J], f32, tag="th")
            nc.sync.dma_start(th_init[:], th0_d[:])
            om = sp.tile([B, J], f32, tag="om")
            nc.sync.dma_start(om[:], om0_d[:])

            th_ap = th_init[:]

            # ---- time loop (fully unrolled) ------------------------------
            for t in range(T_run):
                xt = xp.tile([IN + 1, B], mv_dt, tag="xt")
                nc.sync.dma_start(xt[:], xT_d[:, t, :])

                # ===== layer 0 gates =====
                # chunk-major order: consecutive matmuls share the stationary
                # operand so walrus ldw-opt can elide redundant LDWEIGHTS.
                ps_rz = prz.tile([B, 1024], f32, tag="rz")
                ps_ni = pni.tile([B, 1024], f32, tag="ni")
                hn = ps_ni[:, 0:512]
                inn = ps_ni[:, 512:1024]
                for c in range(4):
                    st = h0T[:, c * B : (c + 1) * B]
                    for ns in range(2):
                        mm(ps_rz[:, ns * 512 : (ns + 1) * 512], st,
                           w0h[:, c, ns * 512 : ns * 512 + 512],
                           start=(c == 0), stop=False)
                    mm(hn, st, w0h[:, c, 1024:1536], start=(c == 0), stop=False)
                for ns in range(2):
                    mm(ps_rz[:, ns * 512 : (ns + 1) * 512], xt[:],
                       w0x[:, ns * 512 : ns * 512 + 512], start=False, stop=True)
                mm(inn, xt[:], w0x[:, 1024:1536], start=True, stop=True)
                mm(hn, brows[:, 2576 : 2576 + B], brows[:, OB0HN : OB0HN + 512],
                   start=False, stop=True)

                # ===== layer 0 pointwise =====
                rz_s = gp.tile([B, 1024], f32, tag="rzs")
                nc.scalar.activation(rz_s[:], ps_rz[:], AF.Sigmoid)
                t0 = gp.tile([B, H], f32, tag="t0")
                nc.vector.tensor_mul(t0[:], rz_s[:, 0:512], hn)
                u0 = gp.tile([B, H], f32, tag="u0")
                nc.vector.tensor_add(u0[:], t0[:], inn)
                n0 = gp.tile([B, H], f32, tag="n0")
                nc.scalar.activation(n0[:], u0[:], AF.Tanh)
                d0 = gp.tile([B, H], f32, tag="d0")
                nc.vector.tensor_sub(d0[:], h0b[:], n0[:])
                zd0 = gp.tile([B, H], f32, tag="zd0")
                nc.vector.tensor_mul(zd0[:], rz_s[:, 512:1024], d0[:])
                h0b = sp.tile([B, H], f32, tag="h0b")
                nc.vector.tensor_add(h0b[:], n0[:], zd0[:])

                # transpose h0' -> h0T'
                ps_hT = pht.tile([128, 4 * B], f32, tag="hT")
                for c in range(4):
                    nc.tensor.transpose(
                        ps_hT[:, c * B : (c + 1) * B],
                        h0b[:, c * 128 : (c + 1) * 128], ident[:],
                    )
                h0T = sp.tile([128, 4 * B], mv_dt, tag="h0T")
                nc.scalar.activation(h0T[:], ps_hT[:], AF.Copy)

                # ===== layer 1 gates =====
                ps_rz1 = prz.tile([B, 1024], f32, tag="rz")
                ps_ni1 = pni.tile([B, 1024], f32, tag="ni")
                hn1 = ps_ni1[:, 0:512]
                inn1 = ps_ni1[:, 512:1024]
                for c in range(4):  # hh part first (h1T ready from t-1)
                    st = h1T[:, c * B : (c + 1) * B]
                    for ns in range(2):
                        mm(ps_rz1[:, ns * 512 : (ns + 1) * 512], st,
                           w1h[:, c, ns * 512 : ns * 512 + 512],
                           start=(c == 0), stop=False)
                    mm(hn1, st, w1h[:, c, 1024:1536], start=(c == 0), stop=False)
                for c in range(4):
                    st = h0T[:, c * B : (c + 1) * B]
                    for ns in range(2):
                        mm(ps_rz1[:, ns * 512 : (ns + 1) * 512], st,
                           w1i[:, c, ns * 512 : ns * 512 + 512],
                           start=False, stop=False)
                    mm(inn1, st, w1i[:, c, 1024:1536], start=(c == 0), stop=False)
                onesap = brows[:, 2576 : 2576 + B]
                for ns in range(2):
                    mm(ps_rz1[:, ns * 512 : (ns + 1) * 512], onesap,
                       brows[:, OB1RZ + ns * 512 : OB1RZ + ns * 512 + 512],
                       start=False, stop=True)
                mm(hn1, onesap, brows[:, OB1HN : OB1HN + 512], start=False, stop=True)
                mm(inn1, onesap, brows[:, OB1IN : OB1IN + 512], start=False, stop=True)

                # ===== layer 1 pointwise =====
                rz1_s = gp.tile([B, 1024], f32, tag="rz1s")
                nc.scalar.activation(rz1_s[:], ps_rz1[:], AF.Sigmoid)
                t1 = gp.tile([B, H], f32, tag="t1")
                nc.vector.tensor_mul(t1[:], rz1_s[:, 0:512], hn1)
                u1 = gp.tile([B, H], f32, tag="u1")
                nc.vector.tensor_add(u1[:], t1[:], inn1)
                n1 = gp.tile([B, H], f32, tag="n1")
                nc.scalar.activation(n1[:], u1[:], AF.Tanh)
                d1 = gp.tile([B, H], f32, tag="d1")
                nc.vector.tensor_sub(d1[:], h1b[:], n1[:])
                zd1 = gp.tile([B, H], f32, tag="zd1")
                nc.vector.tensor_mul(zd1[:], rz1_s[:, 512:1024], d1[:])
                h1b = sp.tile([B, H], f32, tag="h1b")
                nc.vector.tensor_add(h1b[:], n1[:], zd1[:])

                ps_hT1 = pht.tile([128, 4 * B], f32, tag="hT")
                for c in range(4):
                    nc.tensor.transpose(
                        ps_hT1[:, c * B : (c + 1) * B],
                        h1b[:, c * 128 : (c + 1) * 128], ident[:],
                    )
                h1T = sp.tile([128, 4 * B], mv_dt, tag="h1T")
                nc.scalar.activation(h1T[:], ps_hT1[:], AF.Copy)

                # ===== FC + sigmoid =====
                ps_fc = pht.tile([128, 4 * B], f32, tag="hT")
                fc = ps_fc[0:B, 0 : 2 * J]
                for c in range(4):
                    mm(fc, h1T[:, c * B : (c + 1) * B], wfc[:, c, :],
                                     start=(c == 0), stop=False)
                mm(fc, brows[:, 2576 : 2576 + B], brows[:, OBFC : OBFC + 2 * J],
                                 start=False, stop=True)
                a_s = bp.tile([B, J, 2], f32, tag="as")
                nc.scalar.activation(a_s[:], fc, AF.Sigmoid)

                # ===== biophysics (9 fused DVE ops) =====
                a0 = a_s[:, :, 0]
                a1 = a_s[:, :, 1]
                s_ = bp.tile([B, J], f32, tag="s")
                nc.gpsimd.tensor_add(s_[:], a1, a0)
                dd = bp.tile([B, J], f32, tag="dd")
                nc.gpsimd.tensor_sub(dd[:], a1, a0)
                p_ = bp.tile([B, J], f32, tag="p")
                nc.gpsimd.tensor_mul(p_[:], s_[:], dd[:])
                w_ = bp.tile([B, J], f32, tag="w")
                nc.vector.scalar_tensor_tensor(
                    w_[:], dd[:], BETA / GAMMA, p_[:], Alu.mult, Alu.add)
                v_ = bp.tile([B, J], f32, tag="v")
                nc.vector.tensor_scalar(v_[:], s_[:], EPS, DELTA, Alu.mult, Alu.add)
                u2 = bp.tile([B, J], f32, tag="u2")
                nc.vector.tensor_mul(u2[:], v_[:], th_ap)
                q_ = bp.tile([B, J], f32, tag="q")
                nc.vector.scalar_tensor_tensor(
                    q_[:], w_[:], GAMMA, u2[:], Alu.mult, Alu.add)
                om_new = sp.tile([B, J], f32, tag="om")
                nc.vector.scalar_tensor_tensor(
                    om_new[:], om[:], ALPHA, q_[:], Alu.mult, Alu.add)
                om = om_new
                th_new = out_sb[:, t * J : (t + 1) * J]
                nc.vector.scalar_tensor_tensor(
                    th_new, om[:], DT, th_ap, Alu.mult, Alu.add)
                th_ap = th_new

            nc.sync.dma_start(out_d[:], out_sb[:])
    return nc


_NC_CACHE = {}


def _get_nc(T_run):
    key = T_run
    if key in _NC_CACHE:
        return _NC_CACHE[key]
    from concourse import bass, bacc, tile

    mybir = bass.mybir
    mv_dt = mybir.dt.float32r  # fp32 data, 1 cycle/row streaming for N>=256
    nc = bacc.Bacc(None, target_bir_lowering=False)
    _build(nc, bass, tile, mybir, T_run, mv_dt)
    nc.compile()
    _NC_CACHE[key] = nc
    return nc


def _prep_inputs(x, W_ih0, W_hh0, b_ih0, b_hh0, W_ih1, W_hh1, b_ih1, b_hh1,
                 fc_W, fc_b, h0, theta0, omega0):
    T_run = x.shape[1]
    f = np.float32
    xT = np.concatenate(
        [np.ascontiguousarray(x.transpose(2, 1, 0)),
         np.ones((1, T_run, B), f)], axis=0).astype(f)  # [17, T, 64]
    b0rz = (b_ih0 + b_hh0)[:1024]
    w0x = np.concatenate(
        [W_ih0.T, np.concatenate([b0rz, b_ih0[1024:]])[None, :]], axis=0
    ).astype(f)  # [17, 1536]
    w0h = np.ascontiguousarray(W_hh0.T.reshape(4, 128, HG)).astype(f)
    w1i = np.ascontiguousarray(W_ih1.T.reshape(4, 128, HG)).astype(f)
    w1h = np.ascontiguousarray(W_hh1.T.reshape(4, 128, HG)).astype(f)
    wfc = np.ascontiguousarray(fc_W.T.reshape(4, 128, 2 * J)).astype(f)
    brows = np.zeros((1, 2576 + B), f)
    brows[0, 2576:] = 1.0
    brows[0, 0:1024] = (b_ih1 + b_hh1)[:1024]
    brows[0, 1024:1536] = b_ih1[1024:]
    brows[0, 1536:2048] = b_hh1[1024:]
    brows[0, 2048:2560] = b_hh0[1024:]
    brows[0, 2560:2576] = fc_b
    hT0 = np.ascontiguousarray(
        np.stack([h0[0].T.reshape(4, 128, B), h0[1].T.reshape(4, 128, B)])
    ).astype(f)
    return {
        "xT": xT, "w0x": w0x, "w0h": w0h, "w1i": w1i, "w1h": w1h,
        "wfc": wfc, "brows": brows, "ident": np.eye(B, dtype=f),
        "hb0": h0.astype(f), "hT0": hT0,
        "th0": theta0.astype(f), "om0": omega0.astype(f),
    }


def _install_loud_hook():
    """Surface compile-hook exceptions (XLA otherwise swallows them)."""
    import traceback

    from concourse import bass2jax

    if getattr(bass2jax, "_loud_hook_installed", False):
        return
    orig = bass2jax.neuronx_cc_hook

    def loud(*a, **k):
        try:
            return orig(*a, **k)
        except BaseException:
            traceback.print_exc()
            raise

    bass2jax.neuronx_cc_hook = loud
    bass2jax._loud_hook_installed = True

    # Enable walrus's LDWEIGHTS-dedup pass (concourse hardcodes it off).
    # Our matmuls are chunk-major so consecutive MMs share the stationary
    # operand; the dedup removes ~half the weight-load traffic.
    import os

    if os.environ.get("KERNEL_LDW_OPT", "1") == "1":
        from concourse import bass_utils as _bu

        if not getattr(_bu, "_ldw_patch", False):
            _orig_rc = _bu.run_command

            def _rc(cmd, **kw):
                cmd = [c.replace("--enable-ldw-opt=false", "--enable-ldw-opt=true")
                       if isinstance(c, str) else c for c in cmd]
                return _orig_rc(cmd, **kw)

            _bu.run_command = _rc
            _bu._ldw_patch = True


def run(inputs, **spmd_kwargs):
    from concourse.bass_utils import run_bass_kernel_spmd

    _install_loud_hook()

    inputs = {k: np.asarray(v) for k, v in inputs.items()}
    T_run = inputs["x"].shape[1]
    nc = _get_nc(T_run)
    in_map = _prep_inputs(**inputs)
    res = run_bass_kernel_spmd(nc, [in_map] * 8, core_ids=list(range(8)),
                               **spmd_kwargs)
    out = res.results[0]["out"].reshape(B, T_run, J).astype(np.float32)
    return out, res


def kernel(**inputs):
    return run(inputs)[0]


if __name__ == "__main__":
    rs = np.random.RandomState(0)
    demo = {
        "x": rs.randn(B, 8, IN).astype(np.float32),
        "W_ih0": 0.04 * rs.randn(HG, IN).astype(np.float32),
        "W_hh0": 0.04 * rs.randn(HG, H).astype(np.float32),
        "b_ih0": 0.04 * rs.randn(HG).astype(np.float32),
        "b_hh0": 0.04 * rs.randn(HG).astype(np.float32),
        "W_ih1": 0.04 * rs.randn(HG, H).astype(np.float32),
        "W_hh1": 0.04 * rs.randn(HG, H).astype(np.float32),
        "b_ih1": 0.04 * rs.randn(HG).astype(np.float32),
        "b_hh1": 0.04 * rs.randn(HG).astype(np.float32),
        "fc_W": 0.04 * rs.randn(2 * J, H).astype(np.float32),
        "fc_b": 0.04 * rs.randn(2 * J).astype(np.float32),
        "h0": np.zeros((2, B, H), np.float32),
        "theta0": np.zeros((B, J), np.float32),
        "omega0": np.zeros((B, J), np.float32),
    }
    print(kernel(**demo).shape)
